# revision 78
# baseline (speedup 1.0000x reference)
"""Trainium2 Bass kernel for nn_DiffeqExactTraceAttention.

Strategy: data-parallel over batch B=8 across the 8 NeuronCores (one batch
element per core, attention over N=256 fully local, weights replicated).

Per-core computation (all activations stored transposed, [feat, token]):
  query MADE-MLP -> qT [2048, 256]; k/v tanh-MLPs -> kT, vT [128, 256]
  per dim d (16): scoresT[m,n] per head via PE, exp on ACT (no max needed:
  |scores| < 1), diagonal correction via separately-computed diag scores,
  o + softmax denominator from one matmul against [v' | 1], per-partition
  normalize, PE transpose, projection + dimwise 4-layer MLP forward and
  JVP (diagonal Jacobian).

All matmul/DVE traffic is fp16 (PSUM accumulation stays fp32); weights are
packed into two fp16 SBUF blobs + one fp32 bias blob loaded with a handful
of large DMAs so the weight load stays off the critical path.

Outputs y, jac [B, N, D] (d_b3 added host-side).
"""

import os
import sys
import threading

import numpy as np

sys.path.insert(0, "/opt/trn_rl_repo")

import concourse.bass as bass  # noqa: E402
import concourse.mybir as mybir  # noqa: E402
import concourse.tile as tile  # noqa: E402
from concourse import bacc  # noqa: E402

F32 = mybir.dt.float32
F16 = mybir.dt.float16
AF = mybir.ActivationFunctionType
ALU = mybir.AluOpType

B, N, D = 8, 256, 16
HID, H, DH, NH = 256, 128, 64, 4
dh = H // NH  # 32

_lock = threading.Lock()
_cache = {}

# fp16 pack1 segment offsets (cols)
PK1 = {"w1q": 0, "kw1": 512, "vw1": 1024, "kw2": 1536, "vw2": 1792,
       "pw": 2048, "ident": 2112, "blockones": 2240, "ones1": 2244}
PK1_COLS = 2245
# fp16 pack2 segment offsets
PK2 = {"w2q": 0, "dw1": 4096, "dw1g": 4608, "dw2": 5120, "w3ab": 5376,
       "wc": 5380}
PK2_COLS = 5636
# fp32 bias pack column offsets
PKB = {"qb0": 0, "qb1": 2, "qb2": 4, "kb0": 20, "kb1": 22, "kb2": 24,
       "vb0": 25, "vb1": 27, "vb2": 29, "b0p": 30, "w0xpm": 32,
       "db1": 36, "db2": 38, "hmask": 39}
PKB_COLS = 43


def _emit(nc, tc, ins, out):
    """Emit the per-core kernel. `ins` maps name -> dram AP."""
    from contextlib import ExitStack

    with ExitStack() as ctx:
        cw = ctx.enter_context(tc.tile_pool(name="cw", bufs=1))
        sb1 = ctx.enter_context(tc.tile_pool(name="sb1", bufs=1))
        work = ctx.enter_context(tc.tile_pool(name="work", bufs=3))
        epool = ctx.enter_context(tc.tile_pool(name="epool", bufs=3))
        psum = ctx.enter_context(tc.tile_pool(name="psum", bufs=1, space="PSUM"))

        # ---- HAM warm-up: an uninterrupted matmul burst trips the PE
        # clock gate to 2.4 GHz while the weight DMAs stream; the dense
        # body keeps it warm.
        wdat = sb1.tile([128, 512], F16, tag="wdat")
        nc.vector.memset(wdat, 0.001)
        pwu = psum.tile([128, 512], F32, tag="ps", bufs=4, name="pwu")
        for _ in range(16):
            nc.tensor.matmul(pwu, wdat[:, 0:128], wdat, start=True, stop=True)

        # ---- persistent packs, loaded with a few large DMAs ----
        xpk = cw.tile([16, 1024], F16, tag="xpk")
        pkb = cw.tile([128, PKB_COLS], F32, tag="pkb")
        pk1 = cw.tile([128, PK1_COLS], F16, tag="pk1")
        pk2 = cw.tile([128, PK2_COLS], F16, tag="pk2")
        wxo = cw.tile([2, 256], F16, tag="wxo")
        zx = cw.tile([2, 4096], F16, tag="zx")

        nc.sync.dma_start(out=xpk, in_=ins["xpk"])
        nc.sync.dma_start(out=pkb, in_=ins["pkb"])
        for lo, hi in ((0, 512), (512, 1536), (1536, PK1_COLS)):
            nc.sync.dma_start(out=pk1[:, lo:hi], in_=ins["pk1"][:, lo:hi])
        nc.sync.dma_start(out=wxo, in_=ins["w1z"])
        nc.sync.dma_start(out=zx, in_=ins["xflat"])
        for lo, hi in ((0, 1366), (1366, 2732), (2732, 4096),
                       (4096, 4608), (4608, PK2_COLS)):
            nc.sync.dma_start(out=pk2[:, lo:hi], in_=ins["pk2"][:, lo:hi])

        def v1seg(name):  # pack1 2D view
            s = PK1[name]
            n = {"w1q": 512, "kw1": 512, "vw1": 512, "kw2": 256, "vw2": 256,
                 "pw": 64, "ident": 128, "blockones": 4, "ones1": 1}[name]
            return pk1[:, s:s + n]

        t = {
            "xT": xpk[:, 0:256], "w0q": xpk[:, 256:512],
            "kw0": xpk[:, 512:768], "vw0": xpk[:, 768:1024],
            "w1q": v1seg("w1q").rearrange("p (k m) -> p k m", k=2),
            "kw1": v1seg("kw1").rearrange("p (k m) -> p k m", k=2),
            "vw1": v1seg("vw1").rearrange("p (k m) -> p k m", k=2),
            "kw2": v1seg("kw2").rearrange("p (k m) -> p k m", k=2),
            "vw2": v1seg("vw2").rearrange("p (k m) -> p k m", k=2),
            "pw": v1seg("pw"), "ident": v1seg("ident"),
            "blockones": v1seg("blockones"), "ones1": v1seg("ones1"),
            "w2q": pk2[:, 0:4096].rearrange("p (k m) -> p k m", k=2),
            "dw1": pk2[:, 4096:4608].rearrange("p (k m) -> p k m", k=2),
            "dw1g": pk2[:, 4608:5120].rearrange("p (k m) -> p k m", k=2),
            "dw2": pk2[:, 5120:5376].rearrange("p (k m) -> p k m", k=2),
            "w3ab": pk2[:, 5376:5380],
            "wc": pk2[:, 5380:5636],
            "wxo": wxo,
        }
        for bname in ("qb0", "qb1", "qb2", "kb0", "kb1", "kb2", "vb0",
                      "vb1", "vb2", "b0p", "w0xpm", "db1", "db2", "hmask"):
            s = PKB[bname]
            n = {"qb2": 16, "w0xpm": 4, "hmask": 4}.get(
                bname, 1 if bname in ("kb2", "vb2", "db2") else 2)
            t[bname] = pkb[:, s:s + n]

        # ---- stage 1: query / key / value nets ----
        hq1 = sb1.tile([128, 2, 256], F16, tag="hq1")
        hq2 = sb1.tile([128, 2, 256], F16, tag="hq2")
        qT = sb1.tile([128, 16, 256], F16, tag="qT")
        kT = sb1.tile([128, 256], F16, tag="kT")
        vT = sb1.tile([128, 256], F16, tag="vT")

        # q/k/v stage-1 nets as an interleaved ladder: while one net's
        # activation runs on ACT/DVE, the PE works on another net's layer.
        pq0s = []
        for mc in range(2):
            p = psum.tile([128, 256], F32, tag="ps", bufs=4, name="p1")
            nc.tensor.matmul(p, t["w0q"][:, bass.ts(mc, 128)], t["xT"],
                             start=True, stop=True)
            pq0s.append(p)
        for mc in range(2):
            nc.vector.tensor_scalar(out=hq1[:, mc], in0=pq0s[mc],
                                    scalar1=t["qb0"][:, mc:mc + 1],
                                    scalar2=0.0, op0=ALU.add, op1=ALU.max)

        pkv0 = {}
        for nm, w0 in (("k", "kw0"), ("v", "vw0")):
            p = psum.tile([128, 1024], F32, tag="pscore", bufs=2, name="pkv")
            for mc in range(2):
                nc.tensor.matmul(p[:, bass.ts(mc, 256)],
                                 t[w0][:, bass.ts(mc, 128)], t["xT"],
                                 start=True, stop=True)
            pkv0[nm] = p
        h1kv = {}
        for nm, b0 in (("k", "kb0"), ("v", "vb0")):
            h1 = work.tile([128, 2, 256], F16, tag="kv1")
            for mc in range(2):
                nc.scalar.activation(h1[:, mc],
                                     pkv0[nm][:, bass.ts(mc, 256)],
                                     AF.Tanh, bias=t[b0][:, mc:mc + 1])
            h1kv[nm] = h1

        for mc in range(2):
            p = psum.tile([128, 256], F32, tag="ps", bufs=4, name="p1")
            for kc in range(2):
                nc.tensor.matmul(p, t["w1q"][:, kc, bass.ts(mc, 128)],
                                 hq1[:, kc], start=(kc == 0), stop=(kc == 1))
            nc.vector.tensor_scalar(out=hq2[:, mc], in0=p,
                                    scalar1=t["qb1"][:, mc:mc + 1],
                                    scalar2=0.0, op0=ALU.add, op1=ALU.max)

        pkv1 = {}
        for nm, w1 in (("k", "kw1"), ("v", "vw1")):
            p2 = psum.tile([128, 1024], F32, tag="pscore", bufs=2,
                           name="pkv2")
            for mc in range(2):
                for kc in range(2):
                    nc.tensor.matmul(p2[:, bass.ts(mc, 256)],
                                     t[w1][:, kc, bass.ts(mc, 128)],
                                     h1kv[nm][:, kc], start=(kc == 0),
                                     stop=(kc == 1))
            pkv1[nm] = p2
        h2kv = {}
        for nm, b1 in (("k", "kb1"), ("v", "vb1")):
            h2 = work.tile([128, 2, 256], F16, tag="kv2")
            for mc in range(2):
                nc.scalar.activation(h2[:, mc],
                                     pkv1[nm][:, bass.ts(mc, 256)],
                                     AF.Tanh, bias=t[b1][:, mc:mc + 1])
            h2kv[nm] = h2
        pkv2 = {}
        for nm, w2 in (("k", "kw2"), ("v", "vw2")):
            p = psum.tile([128, 256], F32, tag="ps", bufs=4, name="p1")
            for kc in range(2):
                nc.tensor.matmul(p, t[w2][:, kc, :], h2kv[nm][:, kc],
                                 start=(kc == 0), stop=(kc == 1))
            pkv2[nm] = p
        nc.vector.tensor_scalar_add(out=kT, in0=pkv2["k"],
                                    scalar1=t["kb2"][:, 0:1])
        nc.vector.tensor_scalar_add(out=vT, in0=pkv2["v"],
                                    scalar1=t["vb2"][:, 0:1])

        # v' = vT transposed, augmented with ones cols, both token chunks
        # in one tile: v1b2 [128, mj, 4*33]
        v1b2 = sb1.tile([128, 2, 132], F16, tag="v1b2")
        for mj in range(2):
            pt = psum.tile([128, 128], F16, tag="ps", bufs=4, name="ptv")
            nc.tensor.transpose(pt, vT[:, bass.ts(mj, 128)], t["ident"])
            for hh in range(4):
                nc.vector.tensor_copy(
                    out=v1b2[:, mj, hh * 33:hh * 33 + 32],
                    in_=pt[:, bass.ts(hh, 32)])
        ones_view = v1b2.rearrange("p m (h t) -> p m h t", t=33)[:, :, :,
                                                                32:33]
        nc.vector.tensor_copy(out=ones_view,
                              in_=t["ones1"].to_broadcast([128, 2, 4, 1]))

        # q output layer (MADE layer 2), fp16 weights from pack2
        def ql2(g):
            p = psum.tile([128, 1024], F32, tag="pscore", bufs=2, name="pq")
            for sub in range(4):
                mc = g * 4 + sub
                for kc in range(2):
                    nc.tensor.matmul(p[:, bass.ts(sub, 256)],
                                     t["w2q"][:, kc, bass.ts(mc, 128)],
                                     hq2[:, kc], start=(kc == 0),
                                     stop=(kc == 1))
            qb2v = bass.AP(tensor=pkb.tensor,
                           offset=pkb.offset + PKB["qb2"] + g * 4,
                           ap=[[PKB_COLS, 128], [1, 4], [0, 256]])
            nc.vector.tensor_add(qT[:, g * 4:(g + 1) * 4, :],
                                 p.rearrange("p (s n) -> p s n", n=256), qb2v)

        # Diagonal scores per q-layer2 group: prodAll[.,d,.] = qT[.,d,.]*kT,
        # snn[n, 4h] per n-chunk via block-ones matmul, exp per chunk.
        # Chunked so the pipeline can start before all of qT exists.
        prodAll = sb1.tile([128, 16, 256], F16, tag="prodAll")
        edA = sb1.tile([128, 2, 64], F16, tag="edA")

        def prep():
            kTb = bass.AP(tensor=kT.tensor, offset=kT.offset,
                          ap=[[256, 128], [0, 16], [1, 256]])
            nc.vector.tensor_mul(prodAll, qT, kTb)
            for ns in range(2):
                psn = psum.tile([128, 64], F32, tag="ps", bufs=4, name="psn")
                for d_ in range(D):
                    nc.tensor.matmul(psn[:, d_ * 4:(d_ + 1) * 4],
                                     prodAll[:, d_, bass.ts(ns, 128)],
                                     t["blockones"], start=True, stop=True)
                nc.scalar.activation(edA[:, ns], psn, AF.Exp)

        # ---- stage 2: software-pipelined attention + dimwise ----
        # Every PE consumer lags >=1 step behind its producer chain so the
        # in-order PE stream never waits on ACT/DVE latency: per step d we
        # emit scores(d), mm_o(d-1), transposes/hfeat(d-3), and staggered
        # dimwise layer stages. A dense PE stream keeps the HAM clock warm.
        yj = sb1.tile([2, D * N], F32, tag="yj")
        es_st = {}
        oA_st = {}
        oTd2_st = {}
        ag1_st = {}
        ag2_st = {}
        ag3_st = {}

        def _sc2_mj(p_, mj, es):
            for half in range(2):
                ps = psum.tile([128, 1024], F32, tag="pscore", bufs=2,
                               name="pscore")
                for hi in range(2):
                    hh = half * 2 + hi
                    # per-head 32-row contraction placed at PE array
                    # row 32*hh via tile_position (smaller LDWEIGHTS,
                    # no masked kT copies needed)
                    nc.tensor.matmul(
                        ps[:, bass.ts(hi, 512)],
                        kT[32 * hh:32 * (hh + 1), bass.ts(mj, 128)],
                        qT[32 * hh:32 * (hh + 1), 2 * p_:2 * p_ + 2, :],
                        start=True, stop=True,
                        tile_position=(32 * hh, 0))
                e = epool.tile([128, 1024], F16, tag="e",
                               bufs=9, name="e")
                nc.scalar.activation(e, ps, AF.Exp)
                es[(mj, half)] = e

        def SC2a(p_):
            # first token-chunk of the scores: top of the iteration
            es = {}
            _sc2_mj(p_, 0, es)
            es_st[p_] = es

        def SC2b(p_):
            # second chunk at the iteration's END, so its pscore tiles
            # never make the in-order PE wait on the first chunk's exps
            _sc2_mj(p_, 1, es_st[p_])

        def SC2(p_):
            SC2a(p_)
            SC2b(p_)

        def MO(d):
            p_, di = d // 2, d % 2
            es = es_st[p_]
            # diag-correction terms v1*ed in ONE DVE op, emitted first:
            # they depend only on stage-1 outputs, so they overlap the poA
            # matmuls below.
            oAc = work.tile([128, 264], F16, tag="oAc", bufs=4, name="oAc")
            oAcv = oAc.rearrange("p (ns h t) -> p ns h t", ns=2, t=33)
            for ns in range(2):
                nc.vector.tensor_mul(
                    oAcv[:, ns:ns + 1],
                    v1b2[:, ns].rearrange("p (h t) -> p h t",
                                          t=33).unsqueeze(1),
                    bass.AP(tensor=edA.tensor,
                            offset=edA.offset + ns * 64 + d * 4,
                            ap=[[128, 128], [1, 1], [1, 4], [0, 33]]))
            poA = psum.tile([128, 264], F32, tag="ps", bufs=4, name="poA")
            for ns in range(2):
                for hh in range(4):
                    for mj in range(2):
                        e = es[(mj, hh // 2)]
                        o0 = (hh % 2) * 512 + di * 256 + ns * 128
                        nc.tensor.matmul(
                            poA[:, ns * 132 + hh * 33:ns * 132 + (hh + 1) * 33],
                            e[:, o0:o0 + 128],
                            v1b2[:, mj, hh * 33:(hh + 1) * 33],
                            start=(mj == 0), stop=(mj == 1))
            if di == 1:
                del es_st[p_]
            # batched diag-correction + normalize over both n-chunks
            nc.vector.tensor_sub(oAc, oAc, poA)
            rinv = work.tile([128, 8], F32, tag="rinv", bufs=4, name="rinv")
            nc.vector.reciprocal(rinv.rearrange("p (ns h) -> p ns h", ns=2),
                                 oAcv[:, :, :, 32:33])
            oA = work.tile([128, 256], F16, tag="oA", bufs=4, name="oA")
            nc.vector.tensor_mul(
                oA.rearrange("p (ns h c) -> p ns h c", ns=2, c=32),
                oAcv[:, :, :, 0:32],
                rinv.rearrange("p (ns h) -> p ns h", ns=2).to_broadcast(
                    [128, 2, 4, 32]))
            oA_st[d] = oA

        def T(d):
            p_, di = d // 2, d % 2
            if di == 0:
                oTd2_st[p_] = work.tile([128, 512], F16, tag="oTd2", bufs=3,
                                        name="oTd2")
            oTd2 = oTd2_st[p_]
            pt = psum.tile([128, 256], F16, tag="ps", bufs=4, name="ptr")
            for ns in range(2):
                nc.tensor.transpose(pt[:, bass.ts(ns, 128)],
                                    oA_st[d][:, bass.ts(ns, 128)],
                                    t["ident"])
            nc.vector.tensor_copy(
                out=oTd2[:, di * 256:(di + 1) * 256], in_=pt)
            del oA_st[d]

        def L1(p_):
            # the p_w projection is folded into the layer-1 weights on the
            # host (Wc = p_w @ w1z_h), so L1 reads oTd2 directly; the x and
            # bias channels come in via a tiny 2-row matmul against zx
            # (row 0 = x, row 1 = ones; bias weights in wxo row 1).
            # g1 = a1^2 - 1; w0x and the sign live in dw1g.
            a1 = work.tile([128, 2, 512], F16, tag="a1", bufs=2, name="a1")
            g1 = work.tile([128, 2, 512], F16, tag="g1", bufs=2, name="g1")
            oTd2 = oTd2_st.pop(p_)
            pdm = psum.tile([128, 1024], F32, tag="pscore", bufs=2,
                            name="pdm1")
            for mc in range(2):
                nc.tensor.matmul(pdm[:, bass.ts(mc, 512)],
                                 t["wc"][:, bass.ts(mc, 128)],
                                 oTd2, start=True, stop=False)
                nc.tensor.matmul(pdm[:, bass.ts(mc, 512)],
                                 t["wxo"][:, bass.ts(mc, 128)],
                                 zx[:, bass.ts(p_, 512)],
                                 start=False, stop=True)
            nc.scalar.activation(a1.rearrange("p a b -> p (a b)"), pdm,
                                 AF.Tanh)
            nc.vector.tensor_mul(g1, a1, a1)
            nc.vector.tensor_scalar(
                out=g1.rearrange("p a b -> p (a b)"),
                in0=g1.rearrange("p a b -> p (a b)"),
                scalar1=1.0, scalar2=-1.0, op0=ALU.mult, op1=ALU.add)
            ag1_st[p_] = (a1, g1)

        def L2(p_):
            a1, g1 = ag1_st.pop(p_)
            a2 = work.tile([128, 2, 512], F16, tag="a2", bufs=2, name="a2")
            g2 = work.tile([128, 2, 512], F16, tag="g2", bufs=2, name="g2")
            pgs = []
            for mc in range(2):
                pa = psum.tile([128, 512], F32, tag="ps", bufs=4, name="pdm")
                pg = psum.tile([128, 512], F32, tag="ps", bufs=4, name="pdg")
                for kc in range(2):
                    nc.tensor.matmul(pa, t["dw1"][:, kc, bass.ts(mc, 128)],
                                     a1[:, kc], start=(kc == 0),
                                     stop=(kc == 1))
                for kc in range(2):
                    nc.tensor.matmul(pg, t["dw1g"][:, kc, bass.ts(mc, 128)],
                                     g1[:, kc], start=(kc == 0),
                                     stop=(kc == 1))
                nc.scalar.activation(a2[:, mc], pa, AF.Tanh,
                                     bias=t["db1"][:, mc:mc + 1])
                pgs.append(pg)
            nc.vector.tensor_mul(g2, a2, a2)
            for mc in range(2):
                nc.vector.scalar_tensor_tensor(
                    out=g2[:, mc], in0=g2[:, mc], scalar=1.0, in1=pgs[mc],
                    op0=ALU.subtract, op1=ALU.mult)
            ag2_st[p_] = (a2, g2)

        def L3(p_):
            a2, g2 = ag2_st.pop(p_)
            a3 = work.tile([128, 512], F16, tag="a3", bufs=2, name="a3")
            g3 = work.tile([128, 512], F16, tag="g3", bufs=2, name="g3")
            pa = psum.tile([128, 512], F32, tag="ps", bufs=4, name="pdm")
            pg = psum.tile([128, 512], F32, tag="ps", bufs=4, name="pdg")
            for kc in range(2):
                nc.tensor.matmul(pa, t["dw2"][:, kc, :], a2[:, kc],
                                 start=(kc == 0), stop=(kc == 1))
            for kc in range(2):
                nc.tensor.matmul(pg, t["dw2"][:, kc, :], g2[:, kc],
                                 start=(kc == 0), stop=(kc == 1))
            nc.scalar.activation(a3, pa, AF.Tanh, bias=t["db2"][:, 0:1])
            nc.vector.tensor_mul(g3, a3, a3)
            nc.vector.scalar_tensor_tensor(
                out=g3, in0=g3, scalar=1.0, in1=pg,
                op0=ALU.subtract, op1=ALU.mult)
            ag3_st[p_] = (a3, g3)

        def L4(p_):
            a3, g3 = ag3_st.pop(p_)
            py = psum.tile([2, 512], F32, tag="ps", bufs=4, name="py")
            nc.tensor.matmul(py, t["w3ab"][:, 0:2], a3, start=True,
                             stop=False)
            nc.tensor.matmul(py, t["w3ab"][:, 2:4], g3, start=False,
                             stop=True)
            nc.vector.tensor_copy(out=yj[:, bass.ts(p_, 512)], in_=py)
            # stream the output out in halves so the final DMA only covers
            # the last quarter of the drain
            if p_ == 3:
                nc.sync.dma_start(out=out[:, 0:2048], in_=yj[:, 0:2048])
            elif p_ == D // 2 - 1:
                nc.sync.dma_start(out=out[:, 2048:4096], in_=yj[:, 2048:4096])

        # schedule: SC(d); MO(d-1); T(d-2); L1(p)@d=2p+4, L2@2p+6,
        # L3@2p+7, L4@2p+8 (L2 lags L1 by a full d-pair so the PE never
        # waits on the a1 -> g1 elementwise chain)
        def dw_stages(d):
            # the last pipeline step uses the tight offsets to shorten the
            # drain tail (no PE contention left by then)
            for stage, off, off_last in ((L1, 4, 4), (L2, 6, 5), (L3, 7, 6),
                                         (L4, 8, 7)):
                for o in {off, off_last}:
                    if d >= o and (d - o) % 2 == 0:
                        p_ = (d - o) // 2
                        if p_ < D // 2 and (
                                (o == off and p_ < D // 2 - 1)
                                or (o == off_last and p_ == D // 2 - 1)):
                            stage(p_)

        # SC2(0) interleaves with the remaining q-layer2 groups so the
        # first exp starts as early as possible; the remaining ql2 groups
        # and diag-score chunks are folded into the pipeline.
        ql2(0)
        SC2(0)
        for g in range(1, 4):
            ql2(g)
        prep()
        for d in range(1, D + 9):
            if d % 2 == 0 and d // 2 < D // 2:
                SC2a(d // 2)
            if 0 <= d - 2 < D:
                T(d - 2)
            if 0 <= d - 1 < D:
                MO(d - 1)
            dw_stages(d)
            if d % 2 == 0 and d // 2 < D // 2:
                SC2b(d // 2)


def _build():
    nc = bacc.Bacc("TRN2", target_bir_lowering=False, debug=False)
    shapes = {
        "xpk": ([16, 1024], F16), "pkb": ([128, PKB_COLS], F32),
        "pk1": ([128, PK1_COLS], F16), "pk2": ([128, PK2_COLS], F16),
        "w1z": ([2, 256], F16), "xflat": ([2, D * N], F16),
    }
    ins = {n: nc.dram_tensor(n, s, dt, kind="ExternalInput").ap()
           for n, (s, dt) in shapes.items()}
    out = nc.dram_tensor("out", [2, D * N], F32, kind="ExternalOutput").ap()
    with tile.TileContext(nc) as tc:
        _emit(nc, tc, ins, out)
    nc.finalize()
    return nc


def _col2(v):
    # [256] -> [128, 2] with column mc = chunk mc
    return np.ascontiguousarray(v.reshape(2, 128).T)


def _prep_inputs(t, x, q_w0, q_b0, k_w0, k_b0, v_w0, v_b0, q_w1, q_b1, k_w1,
                 k_b1, v_w1, v_b1, q_w2, q_b2, k_w2, k_b2, v_w2, v_b2, p_w,
                 p_b, d_w0, d_b0, d_w1, d_b1, d_w2, d_b2, d_w3, d_b3, q_m0,
                 q_m1, q_m2):
    f = np.float32
    f16 = np.float16
    scale = f(1.0 / np.sqrt(dh))

    def kchunk(w):  # [256, M] -> [128, 2*M] (row chunk-major)
        return np.ascontiguousarray(
            w.reshape(2, 128, -1).transpose(1, 0, 2).reshape(128, -1))

    W0q = (q_w0 * q_m0).astype(f)
    W1q = (q_w1 * q_m1).astype(f)
    W2q = (q_w2 * q_m2).astype(f)
    # b0p folds the t-channel AND the p_b projection bias contribution
    b0p = (d_b0 + t[0] * d_w0[0] + p_b @ d_w0[2:66]).astype(f)
    w0x = d_w0[1].astype(f)
    wxo = np.stack([w0x, b0p]).astype(f)
    Wc = (p_w.astype(f) @ d_w0[2:66].astype(f))
    w3ab = np.zeros((H, 4), f)
    w3ab[:, 0] = d_w3[:, 0]
    w3ab[:, 3] = d_w3[:, 0]
    w0xpm = np.concatenate([_col2(-w0x), _col2(w0x)], axis=1)

    pk1 = np.concatenate([
        kchunk(W1q), kchunk(k_w1.astype(f)), kchunk(v_w1.astype(f)),
        kchunk((k_w2 * scale).astype(f)), kchunk(v_w2.astype(f)),
        p_w.astype(f), np.eye(128, dtype=f),
        np.repeat(np.eye(4, dtype=f), 32, axis=0),
        np.ones((128, 1), f),
    ], axis=1).astype(f16)
    pk2 = np.concatenate([
        kchunk(W2q), kchunk(d_w1.astype(f)),
        kchunk((d_w1 * (-w0x[:, None])).astype(f)),
        kchunk(d_w2.astype(f)), w3ab, Wc,
    ], axis=1).astype(f16)
    pkb = np.concatenate([
        _col2(q_b0.astype(f)), _col2(q_b1.astype(f)),
        np.ascontiguousarray(q_b2.astype(f).reshape(16, 128).T),
        _col2(k_b0.astype(f)), _col2(k_b1.astype(f)),
        (k_b2 * scale).astype(f).reshape(128, 1),
        _col2(v_b0.astype(f)), _col2(v_b1.astype(f)),
        v_b2.astype(f).reshape(128, 1),
        _col2(b0p), w0xpm,
        _col2(d_b1.astype(f)), d_b2.astype(f).reshape(128, 1),
        np.repeat(np.eye(4, dtype=f), 32, axis=0),
    ], axis=1)
    shared = {
        "pk1": np.ascontiguousarray(pk1),
        "pk2": np.ascontiguousarray(pk2),
        "pkb": np.ascontiguousarray(pkb),
        "w1z": np.ascontiguousarray(wxo.astype(f16)),
    }
    in_maps = []
    for b in range(B):
        m = dict(shared)
        xt = np.ascontiguousarray(x[b].T.astype(f)).astype(f16)
        m["xpk"] = np.ascontiguousarray(np.concatenate(
            [xt, W0q.astype(f16), k_w0.astype(f).astype(f16),
             v_w0.astype(f).astype(f16)], axis=1))
        m["xflat"] = np.ascontiguousarray(np.concatenate(
            [xt.reshape(1, -1), np.ones((1, D * N), f16)], axis=0))
        in_maps.append(m)
    return in_maps, float(d_b3[0])


def kernel(**inputs):
    from concourse.bass_utils import run_bass_kernel_spmd

    inputs = {k: np.asarray(v) for k, v in inputs.items()}
    with _lock:
        if "nc" not in _cache:
            _cache["nc"] = _build()
        nc = _cache["nc"]
    in_maps, b3 = _prep_inputs(**inputs)
    trace = False
    if os.environ.get("KBENCH_TRACE"):
        try:
            import antenv.axon_hooks  # noqa: F401
            trace = True
        except ImportError:
            trace = False
    res = run_bass_kernel_spmd(nc, in_maps, list(range(B)), trace=trace)
    if trace:
        _cache["last_results"] = res
    y = np.zeros((B, N, D), np.float32)
    jac = np.zeros((B, N, D), np.float32)
    for b in range(B):
        o = res.results[b]["out"].reshape(2, D, N)
        y[b] = o[0].T + np.float32(b3)
        jac[b] = o[1].T
    return y, jac


# revision 79
# speedup vs baseline: 1.0365x; 1.0365x over previous
"""Trainium2 Bass kernel for nn_DiffeqExactTraceAttention.

Strategy: data-parallel over batch B=8 across the 8 NeuronCores (one batch
element per core, attention over N=256 fully local, weights replicated).

Per-core computation (all activations stored transposed, [feat, token]):
  query MADE-MLP -> qT [2048, 256]; k/v tanh-MLPs -> kT, vT [128, 256]
  per dim d (16): scoresT[m,n] per head via PE, exp on ACT (no max needed:
  |scores| < 1), diagonal correction via separately-computed diag scores,
  o + softmax denominator from one matmul against [v' | 1], per-partition
  normalize, PE transpose, projection + dimwise 4-layer MLP forward and
  JVP (diagonal Jacobian).

All matmul/DVE traffic is fp16 (PSUM accumulation stays fp32); weights are
packed into two fp16 SBUF blobs + one fp32 bias blob loaded with a handful
of large DMAs so the weight load stays off the critical path.

Outputs y, jac [B, N, D] (d_b3 added host-side).
"""

import os
import sys
import threading

import numpy as np

sys.path.insert(0, "/opt/trn_rl_repo")

import concourse.bass as bass  # noqa: E402
import concourse.mybir as mybir  # noqa: E402
import concourse.tile as tile  # noqa: E402
from concourse import bacc  # noqa: E402

F32 = mybir.dt.float32
F16 = mybir.dt.float16
AF = mybir.ActivationFunctionType
ALU = mybir.AluOpType

B, N, D = 8, 256, 16
HID, H, DH, NH = 256, 128, 64, 4
dh = H // NH  # 32

_lock = threading.Lock()
_cache = {}

# fp16 pack1 segment offsets (cols)
PK1 = {"w1q": 0, "kw1": 512, "vw1": 1024, "kw2": 1536, "vw2": 1792,
       "pw": 2048, "ident": 2112, "blockones": 2240, "ones1": 2244}
PK1_COLS = 2245
# fp16 pack2 segment offsets
PK2 = {"w2q": 0, "dw1": 4096, "dw1g": 4608, "dw2": 5120, "w3ab": 5376,
       "wc": 5380}
PK2_COLS = 5636
# fp32 bias pack column offsets
PKB = {"qb0": 0, "qb1": 2, "qb2": 4, "kb0": 20, "kb1": 22, "kb2": 24,
       "vb0": 25, "vb1": 27, "vb2": 29, "b0p": 30, "w0xpm": 32,
       "db1": 36, "db2": 38, "hmask": 39}
PKB_COLS = 43


def _emit(nc, tc, ins, out):
    """Emit the per-core kernel. `ins` maps name -> dram AP."""
    from contextlib import ExitStack

    with ExitStack() as ctx:
        cw = ctx.enter_context(tc.tile_pool(name="cw", bufs=1))
        sb1 = ctx.enter_context(tc.tile_pool(name="sb1", bufs=1))
        work = ctx.enter_context(tc.tile_pool(name="work", bufs=3))
        epool = ctx.enter_context(tc.tile_pool(name="epool", bufs=3))
        psum = ctx.enter_context(tc.tile_pool(name="psum", bufs=1, space="PSUM"))

        # ---- HAM warm-up: an uninterrupted matmul burst trips the PE
        # clock gate to 2.4 GHz while the weight DMAs stream; the dense
        # body keeps it warm.
        wdat = sb1.tile([128, 512], F16, tag="wdat")
        nc.vector.memset(wdat, 0.001)
        pwu = psum.tile([128, 512], F32, tag="ps", bufs=4, name="pwu")
        for _ in range(16):
            nc.tensor.matmul(pwu, wdat[:, 0:128], wdat, start=True, stop=True)

        # ---- persistent packs, loaded with a few large DMAs ----
        xpk = cw.tile([16, 1024], F16, tag="xpk")
        pkb = cw.tile([128, PKB_COLS], F32, tag="pkb")
        pk1 = cw.tile([128, PK1_COLS], F16, tag="pk1")
        pk2 = cw.tile([128, PK2_COLS], F16, tag="pk2")
        wxo = cw.tile([2, 256], F16, tag="wxo")
        zx = cw.tile([2, 4096], F16, tag="zx")

        nc.sync.dma_start(out=xpk, in_=ins["xpk"])
        nc.sync.dma_start(out=pkb, in_=ins["pkb"])
        for lo, hi in ((0, 512), (512, 1536), (1536, PK1_COLS)):
            nc.sync.dma_start(out=pk1[:, lo:hi], in_=ins["pk1"][:, lo:hi])
        nc.sync.dma_start(out=wxo, in_=ins["w1z"])
        nc.sync.dma_start(out=zx, in_=ins["xflat"])
        for lo, hi in ((0, 1366), (1366, 2732), (2732, 4096),
                       (4096, 4608), (4608, PK2_COLS)):
            nc.sync.dma_start(out=pk2[:, lo:hi], in_=ins["pk2"][:, lo:hi])

        def v1seg(name):  # pack1 2D view
            s = PK1[name]
            n = {"w1q": 512, "kw1": 512, "vw1": 512, "kw2": 256, "vw2": 256,
                 "pw": 64, "ident": 128, "blockones": 4, "ones1": 1}[name]
            return pk1[:, s:s + n]

        t = {
            "xT": xpk[:, 0:256], "w0q": xpk[:, 256:512],
            "kw0": xpk[:, 512:768], "vw0": xpk[:, 768:1024],
            "w1q": v1seg("w1q").rearrange("p (k m) -> p k m", k=2),
            "kw1": v1seg("kw1").rearrange("p (k m) -> p k m", k=2),
            "vw1": v1seg("vw1").rearrange("p (k m) -> p k m", k=2),
            "kw2": v1seg("kw2").rearrange("p (k m) -> p k m", k=2),
            "vw2": v1seg("vw2").rearrange("p (k m) -> p k m", k=2),
            "pw": v1seg("pw"), "ident": v1seg("ident"),
            "blockones": v1seg("blockones"), "ones1": v1seg("ones1"),
            "w2q": pk2[:, 0:4096].rearrange("p (k m) -> p k m", k=2),
            "dw1": pk2[:, 4096:4608].rearrange("p (k m) -> p k m", k=2),
            "dw1g": pk2[:, 4608:5120].rearrange("p (k m) -> p k m", k=2),
            "dw2": pk2[:, 5120:5376].rearrange("p (k m) -> p k m", k=2),
            "w3ab": pk2[:, 5376:5380],
            "wc": pk2[:, 5380:5636],
            "wxo": wxo,
        }
        for bname in ("qb0", "qb1", "qb2", "kb0", "kb1", "kb2", "vb0",
                      "vb1", "vb2", "b0p", "w0xpm", "db1", "db2", "hmask"):
            s = PKB[bname]
            n = {"qb2": 16, "w0xpm": 4, "hmask": 4}.get(
                bname, 1 if bname in ("kb2", "vb2", "db2") else 2)
            t[bname] = pkb[:, s:s + n]

        # ---- stage 1: query / key / value nets ----
        hq1 = sb1.tile([128, 2, 256], F16, tag="hq1")
        hq2 = sb1.tile([128, 2, 256], F16, tag="hq2")
        qT = sb1.tile([128, 16, 256], F16, tag="qT")
        kT = sb1.tile([128, 256], F16, tag="kT")
        vT = sb1.tile([128, 256], F16, tag="vT")

        # q/k/v stage-1 nets as an interleaved ladder: while one net's
        # activation runs on ACT/DVE, the PE works on another net's layer.
        pq0s = []
        for mc in range(2):
            p = psum.tile([128, 256], F32, tag="ps", bufs=4, name="p1")
            nc.tensor.matmul(p, t["w0q"][:, bass.ts(mc, 128)], t["xT"],
                             start=True, stop=True)
            pq0s.append(p)
        for mc in range(2):
            nc.vector.tensor_scalar(out=hq1[:, mc], in0=pq0s[mc],
                                    scalar1=t["qb0"][:, mc:mc + 1],
                                    scalar2=0.0, op0=ALU.add, op1=ALU.max)

        pkv0 = {}
        for nm, w0 in (("k", "kw0"), ("v", "vw0")):
            p = psum.tile([128, 1024], F32, tag="pscore", bufs=2, name="pkv")
            for mc in range(2):
                nc.tensor.matmul(p[:, bass.ts(mc, 256)],
                                 t[w0][:, bass.ts(mc, 128)], t["xT"],
                                 start=True, stop=True)
            pkv0[nm] = p
        h1kv = {}
        for nm, b0 in (("k", "kb0"), ("v", "vb0")):
            h1 = work.tile([128, 2, 256], F16, tag="kv1")
            for mc in range(2):
                nc.scalar.activation(h1[:, mc],
                                     pkv0[nm][:, bass.ts(mc, 256)],
                                     AF.Tanh, bias=t[b0][:, mc:mc + 1])
            h1kv[nm] = h1

        for mc in range(2):
            p = psum.tile([128, 256], F32, tag="ps", bufs=4, name="p1")
            for kc in range(2):
                nc.tensor.matmul(p, t["w1q"][:, kc, bass.ts(mc, 128)],
                                 hq1[:, kc], start=(kc == 0), stop=(kc == 1))
            nc.vector.tensor_scalar(out=hq2[:, mc], in0=p,
                                    scalar1=t["qb1"][:, mc:mc + 1],
                                    scalar2=0.0, op0=ALU.add, op1=ALU.max)

        pkv1 = {}
        for nm, w1 in (("k", "kw1"), ("v", "vw1")):
            p2 = psum.tile([128, 1024], F32, tag="pscore", bufs=2,
                           name="pkv2")
            for mc in range(2):
                for kc in range(2):
                    nc.tensor.matmul(p2[:, bass.ts(mc, 256)],
                                     t[w1][:, kc, bass.ts(mc, 128)],
                                     h1kv[nm][:, kc], start=(kc == 0),
                                     stop=(kc == 1))
            pkv1[nm] = p2
        h2kv = {}
        for nm, b1 in (("k", "kb1"), ("v", "vb1")):
            h2 = work.tile([128, 2, 256], F16, tag="kv2")
            for mc in range(2):
                nc.scalar.activation(h2[:, mc],
                                     pkv1[nm][:, bass.ts(mc, 256)],
                                     AF.Tanh, bias=t[b1][:, mc:mc + 1])
            h2kv[nm] = h2
        pkv2 = {}
        for nm, w2 in (("k", "kw2"), ("v", "vw2")):
            p = psum.tile([128, 256], F32, tag="ps", bufs=4, name="p1")
            for kc in range(2):
                nc.tensor.matmul(p, t[w2][:, kc, :], h2kv[nm][:, kc],
                                 start=(kc == 0), stop=(kc == 1))
            pkv2[nm] = p
        nc.vector.tensor_scalar_add(out=kT, in0=pkv2["k"],
                                    scalar1=t["kb2"][:, 0:1])
        nc.vector.tensor_scalar_add(out=vT, in0=pkv2["v"],
                                    scalar1=t["vb2"][:, 0:1])

        # v' = vT transposed, augmented with ones cols, both token chunks
        # in one tile: v1b2 [128, mj, 4*33]
        v1b2 = sb1.tile([128, 2, 132], F16, tag="v1b2")
        for mj in range(2):
            pt = psum.tile([128, 128], F16, tag="ps", bufs=4, name="ptv")
            nc.tensor.transpose(pt, vT[:, bass.ts(mj, 128)], t["ident"])
            for hh in range(4):
                nc.vector.tensor_copy(
                    out=v1b2[:, mj, hh * 33:hh * 33 + 32],
                    in_=pt[:, bass.ts(hh, 32)])
        ones_view = v1b2.rearrange("p m (h t) -> p m h t", t=33)[:, :, :,
                                                                32:33]
        nc.vector.tensor_copy(out=ones_view,
                              in_=t["ones1"].to_broadcast([128, 2, 4, 1]))

        # q output layer (MADE layer 2), fp16 weights from pack2
        def ql2(g):
            p = psum.tile([128, 1024], F32, tag="pscore", bufs=2, name="pq")
            for sub in range(4):
                mc = g * 4 + sub
                for kc in range(2):
                    nc.tensor.matmul(p[:, bass.ts(sub, 256)],
                                     t["w2q"][:, kc, bass.ts(mc, 128)],
                                     hq2[:, kc], start=(kc == 0),
                                     stop=(kc == 1))
            qb2v = bass.AP(tensor=pkb.tensor,
                           offset=pkb.offset + PKB["qb2"] + g * 4,
                           ap=[[PKB_COLS, 128], [1, 4], [0, 256]])
            nc.vector.tensor_add(qT[:, g * 4:(g + 1) * 4, :],
                                 p.rearrange("p (s n) -> p s n", n=256), qb2v)

        # Diagonal scores per q-layer2 group: prodAll[.,d,.] = qT[.,d,.]*kT,
        # snn[n, 4h] per n-chunk via block-ones matmul, exp per chunk.
        # Chunked so the pipeline can start before all of qT exists.
        prodAll = sb1.tile([128, 16, 256], F16, tag="prodAll")
        edA = sb1.tile([128, 2, 64], F16, tag="edA")

        def prep():
            kTb = bass.AP(tensor=kT.tensor, offset=kT.offset,
                          ap=[[256, 128], [0, 16], [1, 256]])
            nc.vector.tensor_mul(prodAll, qT, kTb)
            for ns in range(2):
                psn = psum.tile([128, 64], F32, tag="ps", bufs=4, name="psn")
                for d_ in range(D):
                    nc.tensor.matmul(psn[:, d_ * 4:(d_ + 1) * 4],
                                     prodAll[:, d_, bass.ts(ns, 128)],
                                     t["blockones"], start=True, stop=True)
                nc.scalar.activation(edA[:, ns], psn, AF.Exp)

        # ---- stage 2: software-pipelined attention + dimwise ----
        # Every PE consumer lags >=1 step behind its producer chain so the
        # in-order PE stream never waits on ACT/DVE latency: per step d we
        # emit scores(d), mm_o(d-1), transposes/hfeat(d-3), and staggered
        # dimwise layer stages. A dense PE stream keeps the HAM clock warm.
        yj = sb1.tile([2, D * N], F32, tag="yj")
        es_st = {}
        oA_st = {}
        oTd2_st = {}
        ag1_st = {}
        ag2_st = {}
        ag3_st = {}

        def _sc2_mj(p_, mj, es):
            for half in range(2):
                ps = psum.tile([128, 1024], F32, tag="pscore", bufs=2,
                               name="pscore")
                for hi in range(2):
                    hh = half * 2 + hi
                    # per-head 32-row contraction placed at PE array
                    # row 32*hh via tile_position (smaller LDWEIGHTS,
                    # no masked kT copies needed)
                    nc.tensor.matmul(
                        ps[:, bass.ts(hi, 512)],
                        kT[32 * hh:32 * (hh + 1), bass.ts(mj, 128)],
                        qT[32 * hh:32 * (hh + 1), 2 * p_:2 * p_ + 2, :],
                        start=True, stop=True,
                        tile_position=(32 * hh, 0))
                e = epool.tile([128, 1024], F16, tag="e",
                               bufs=9, name="e")
                nc.scalar.activation(e, ps, AF.Exp)
                es[(mj, half)] = e

        def SC2a(p_):
            # first token-chunk of the scores: top of the iteration
            es = {}
            _sc2_mj(p_, 0, es)
            es_st[p_] = es

        def SC2b(p_):
            # second chunk at the iteration's END, so its pscore tiles
            # never make the in-order PE wait on the first chunk's exps
            _sc2_mj(p_, 1, es_st[p_])

        def SC2(p_):
            SC2a(p_)
            SC2b(p_)

        def MO(d):
            p_, di = d // 2, d % 2
            es = es_st[p_]
            # diag-correction terms v1*ed in ONE DVE op, emitted first:
            # they depend only on stage-1 outputs, so they overlap the poA
            # matmuls below.
            oAc = work.tile([128, 264], F16, tag="oAc", bufs=4, name="oAc")
            oAcv = oAc.rearrange("p (ns h t) -> p ns h t", ns=2, t=33)
            for ns in range(2):
                nc.vector.tensor_mul(
                    oAcv[:, ns:ns + 1],
                    v1b2[:, ns].rearrange("p (h t) -> p h t",
                                          t=33).unsqueeze(1),
                    bass.AP(tensor=edA.tensor,
                            offset=edA.offset + ns * 64 + d * 4,
                            ap=[[128, 128], [1, 1], [1, 4], [0, 33]]))
            poA = psum.tile([128, 264], F32, tag="ps", bufs=4, name="poA")
            for ns in range(2):
                for hh in range(4):
                    for mj in range(2):
                        e = es[(mj, hh // 2)]
                        o0 = (hh % 2) * 512 + di * 256 + ns * 128
                        nc.tensor.matmul(
                            poA[:, ns * 132 + hh * 33:ns * 132 + (hh + 1) * 33],
                            e[:, o0:o0 + 128],
                            v1b2[:, mj, hh * 33:(hh + 1) * 33],
                            start=(mj == 0), stop=(mj == 1))
            if di == 1:
                del es_st[p_]
            # batched diag-correction + normalize over both n-chunks
            nc.vector.tensor_sub(oAc, oAc, poA)
            rinv = work.tile([128, 8], F32, tag="rinv", bufs=4, name="rinv")
            nc.vector.reciprocal(rinv.rearrange("p (ns h) -> p ns h", ns=2),
                                 oAcv[:, :, :, 32:33])
            oA = work.tile([128, 256], F16, tag="oA", bufs=4, name="oA")
            nc.vector.tensor_mul(
                oA.rearrange("p (ns h c) -> p ns h c", ns=2, c=32),
                oAcv[:, :, :, 0:32],
                rinv.rearrange("p (ns h) -> p ns h", ns=2).to_broadcast(
                    [128, 2, 4, 32]))
            oA_st[d] = oA

        def T(d):
            p_, di = d // 2, d % 2
            if di == 0:
                oTd2_st[p_] = work.tile([128, 512], F16, tag="oTd2", bufs=3,
                                        name="oTd2")
            oTd2 = oTd2_st[p_]
            pt = psum.tile([128, 256], F16, tag="ps", bufs=4, name="ptr")
            for ns in range(2):
                nc.tensor.transpose(pt[:, bass.ts(ns, 128)],
                                    oA_st[d][:, bass.ts(ns, 128)],
                                    t["ident"])
            nc.vector.tensor_copy(
                out=oTd2[:, di * 256:(di + 1) * 256], in_=pt)
            del oA_st[d]

        def L1(p_):
            # the p_w projection is folded into the layer-1 weights on the
            # host (Wc = p_w @ w1z_h), so L1 reads oTd2 directly; the x and
            # bias channels come in via a tiny 2-row matmul against zx
            # (row 0 = x, row 1 = ones; bias weights in wxo row 1).
            # g1 = a1^2 - 1; w0x and the sign live in dw1g.
            a1 = work.tile([128, 2, 512], F16, tag="a1", bufs=2, name="a1")
            g1 = work.tile([128, 2, 512], F16, tag="g1", bufs=2, name="g1")
            oTd2 = oTd2_st.pop(p_)
            pdm = psum.tile([128, 1024], F32, tag="pscore", bufs=2,
                            name="pdm1")
            for mc in range(2):
                nc.tensor.matmul(pdm[:, bass.ts(mc, 512)],
                                 t["wc"][:, bass.ts(mc, 128)],
                                 oTd2, start=True, stop=False)
                nc.tensor.matmul(pdm[:, bass.ts(mc, 512)],
                                 t["wxo"][:, bass.ts(mc, 128)],
                                 zx[:, bass.ts(p_, 512)],
                                 start=False, stop=True)
            nc.scalar.activation(a1.rearrange("p a b -> p (a b)"), pdm,
                                 AF.Tanh)
            nc.vector.tensor_mul(g1, a1, a1)
            nc.vector.tensor_scalar(
                out=g1.rearrange("p a b -> p (a b)"),
                in0=g1.rearrange("p a b -> p (a b)"),
                scalar1=1.0, scalar2=-1.0, op0=ALU.mult, op1=ALU.add)
            ag1_st[p_] = (a1, g1)

        def L2(p_):
            a1, g1 = ag1_st.pop(p_)
            a2 = work.tile([128, 2, 512], F16, tag="a2", bufs=2, name="a2")
            g2 = work.tile([128, 2, 512], F16, tag="g2", bufs=2, name="g2")
            pgs = []
            for mc in range(2):
                pa = psum.tile([128, 512], F32, tag="ps", bufs=4, name="pdm")
                pg = psum.tile([128, 512], F32, tag="ps", bufs=4, name="pdg")
                for kc in range(2):
                    nc.tensor.matmul(pa, t["dw1"][:, kc, bass.ts(mc, 128)],
                                     a1[:, kc], start=(kc == 0),
                                     stop=(kc == 1))
                for kc in range(2):
                    nc.tensor.matmul(pg, t["dw1g"][:, kc, bass.ts(mc, 128)],
                                     g1[:, kc], start=(kc == 0),
                                     stop=(kc == 1))
                nc.scalar.activation(a2[:, mc], pa, AF.Tanh,
                                     bias=t["db1"][:, mc:mc + 1])
                pgs.append(pg)
            nc.vector.tensor_mul(g2, a2, a2)
            for mc in range(2):
                nc.vector.scalar_tensor_tensor(
                    out=g2[:, mc], in0=g2[:, mc], scalar=1.0, in1=pgs[mc],
                    op0=ALU.subtract, op1=ALU.mult)
            ag2_st[p_] = (a2, g2)

        def L3(p_):
            a2, g2 = ag2_st.pop(p_)
            a3 = work.tile([128, 512], F16, tag="a3", bufs=2, name="a3")
            g3 = work.tile([128, 512], F16, tag="g3", bufs=2, name="g3")
            pa = psum.tile([128, 512], F32, tag="ps", bufs=4, name="pdm")
            pg = psum.tile([128, 512], F32, tag="ps", bufs=4, name="pdg")
            for kc in range(2):
                nc.tensor.matmul(pa, t["dw2"][:, kc, :], a2[:, kc],
                                 start=(kc == 0), stop=(kc == 1))
            for kc in range(2):
                nc.tensor.matmul(pg, t["dw2"][:, kc, :], g2[:, kc],
                                 start=(kc == 0), stop=(kc == 1))
            nc.scalar.activation(a3, pa, AF.Tanh, bias=t["db2"][:, 0:1])
            nc.vector.tensor_mul(g3, a3, a3)
            nc.vector.scalar_tensor_tensor(
                out=g3, in0=g3, scalar=1.0, in1=pg,
                op0=ALU.subtract, op1=ALU.mult)
            ag3_st[p_] = (a3, g3)

        def L4(p_):
            a3, g3 = ag3_st.pop(p_)
            py = psum.tile([2, 512], F32, tag="ps", bufs=4, name="py")
            nc.tensor.matmul(py, t["w3ab"][:, 0:2], a3, start=True,
                             stop=False)
            nc.tensor.matmul(py, t["w3ab"][:, 2:4], g3, start=False,
                             stop=True)
            nc.vector.tensor_copy(out=yj[:, bass.ts(p_, 512)], in_=py)
            # stream the output out in halves so the final DMA only covers
            # the last quarter of the drain
            if p_ == 3:
                nc.sync.dma_start(out=out[:, 0:2048], in_=yj[:, 0:2048])
            elif p_ == D // 2 - 1:
                nc.sync.dma_start(out=out[:, 2048:4096], in_=yj[:, 2048:4096])

        # schedule: SC(d); MO(d-1); T(d-2); L1(p)@d=2p+4, L2@2p+6,
        # L3@2p+7, L4@2p+8 (L2 lags L1 by a full d-pair so the PE never
        # waits on the a1 -> g1 elementwise chain)
        def dw_stages(d):
            # the last pipeline step uses the tight offsets to shorten the
            # drain tail (no PE contention left by then)
            for stage, off, off_last in ((L1, 4, 4), (L2, 6, 5), (L3, 7, 6),
                                         (L4, 8, 7)):
                for o in {off, off_last}:
                    if d >= o and (d - o) % 2 == 0:
                        p_ = (d - o) // 2
                        if p_ < D // 2 and (
                                (o == off and p_ < D // 2 - 1)
                                or (o == off_last and p_ == D // 2 - 1)):
                            stage(p_)

        # SC2(0) interleaves with the remaining q-layer2 groups so the
        # first exp starts as early as possible; the remaining ql2 groups
        # and diag-score chunks are folded into the pipeline.
        ql2(0)
        SC2(0)
        for g in range(1, 4):
            ql2(g)
        prep()
        for d in range(1, D + 9):
            if d % 2 == 0 and d // 2 < D // 2:
                SC2(d // 2)
            if 0 <= d - 2 < D:
                T(d - 2)
            if 0 <= d - 1 < D:
                MO(d - 1)
            dw_stages(d)


def _build():
    nc = bacc.Bacc("TRN2", target_bir_lowering=False, debug=False)
    shapes = {
        "xpk": ([16, 1024], F16), "pkb": ([128, PKB_COLS], F32),
        "pk1": ([128, PK1_COLS], F16), "pk2": ([128, PK2_COLS], F16),
        "w1z": ([2, 256], F16), "xflat": ([2, D * N], F16),
    }
    ins = {n: nc.dram_tensor(n, s, dt, kind="ExternalInput").ap()
           for n, (s, dt) in shapes.items()}
    out = nc.dram_tensor("out", [2, D * N], F32, kind="ExternalOutput").ap()
    with tile.TileContext(nc) as tc:
        _emit(nc, tc, ins, out)
    nc.finalize()
    return nc


def _col2(v):
    # [256] -> [128, 2] with column mc = chunk mc
    return np.ascontiguousarray(v.reshape(2, 128).T)


def _prep_inputs(t, x, q_w0, q_b0, k_w0, k_b0, v_w0, v_b0, q_w1, q_b1, k_w1,
                 k_b1, v_w1, v_b1, q_w2, q_b2, k_w2, k_b2, v_w2, v_b2, p_w,
                 p_b, d_w0, d_b0, d_w1, d_b1, d_w2, d_b2, d_w3, d_b3, q_m0,
                 q_m1, q_m2):
    f = np.float32
    f16 = np.float16
    scale = f(1.0 / np.sqrt(dh))

    def kchunk(w):  # [256, M] -> [128, 2*M] (row chunk-major)
        return np.ascontiguousarray(
            w.reshape(2, 128, -1).transpose(1, 0, 2).reshape(128, -1))

    W0q = (q_w0 * q_m0).astype(f)
    W1q = (q_w1 * q_m1).astype(f)
    W2q = (q_w2 * q_m2).astype(f)
    # b0p folds the t-channel AND the p_b projection bias contribution
    b0p = (d_b0 + t[0] * d_w0[0] + p_b @ d_w0[2:66]).astype(f)
    w0x = d_w0[1].astype(f)
    wxo = np.stack([w0x, b0p]).astype(f)
    Wc = (p_w.astype(f) @ d_w0[2:66].astype(f))
    w3ab = np.zeros((H, 4), f)
    w3ab[:, 0] = d_w3[:, 0]
    w3ab[:, 3] = d_w3[:, 0]
    w0xpm = np.concatenate([_col2(-w0x), _col2(w0x)], axis=1)

    pk1 = np.concatenate([
        kchunk(W1q), kchunk(k_w1.astype(f)), kchunk(v_w1.astype(f)),
        kchunk((k_w2 * scale).astype(f)), kchunk(v_w2.astype(f)),
        p_w.astype(f), np.eye(128, dtype=f),
        np.repeat(np.eye(4, dtype=f), 32, axis=0),
        np.ones((128, 1), f),
    ], axis=1).astype(f16)
    pk2 = np.concatenate([
        kchunk(W2q), kchunk(d_w1.astype(f)),
        kchunk((d_w1 * (-w0x[:, None])).astype(f)),
        kchunk(d_w2.astype(f)), w3ab, Wc,
    ], axis=1).astype(f16)
    pkb = np.concatenate([
        _col2(q_b0.astype(f)), _col2(q_b1.astype(f)),
        np.ascontiguousarray(q_b2.astype(f).reshape(16, 128).T),
        _col2(k_b0.astype(f)), _col2(k_b1.astype(f)),
        (k_b2 * scale).astype(f).reshape(128, 1),
        _col2(v_b0.astype(f)), _col2(v_b1.astype(f)),
        v_b2.astype(f).reshape(128, 1),
        _col2(b0p), w0xpm,
        _col2(d_b1.astype(f)), d_b2.astype(f).reshape(128, 1),
        np.repeat(np.eye(4, dtype=f), 32, axis=0),
    ], axis=1)
    shared = {
        "pk1": np.ascontiguousarray(pk1),
        "pk2": np.ascontiguousarray(pk2),
        "pkb": np.ascontiguousarray(pkb),
        "w1z": np.ascontiguousarray(wxo.astype(f16)),
    }
    in_maps = []
    for b in range(B):
        m = dict(shared)
        xt = np.ascontiguousarray(x[b].T.astype(f)).astype(f16)
        m["xpk"] = np.ascontiguousarray(np.concatenate(
            [xt, W0q.astype(f16), k_w0.astype(f).astype(f16),
             v_w0.astype(f).astype(f16)], axis=1))
        m["xflat"] = np.ascontiguousarray(np.concatenate(
            [xt.reshape(1, -1), np.ones((1, D * N), f16)], axis=0))
        in_maps.append(m)
    return in_maps, float(d_b3[0])


def kernel(**inputs):
    from concourse.bass_utils import run_bass_kernel_spmd

    inputs = {k: np.asarray(v) for k, v in inputs.items()}
    with _lock:
        if "nc" not in _cache:
            _cache["nc"] = _build()
        nc = _cache["nc"]
    in_maps, b3 = _prep_inputs(**inputs)
    trace = False
    if os.environ.get("KBENCH_TRACE"):
        try:
            import antenv.axon_hooks  # noqa: F401
            trace = True
        except ImportError:
            trace = False
    res = run_bass_kernel_spmd(nc, in_maps, list(range(B)), trace=trace)
    if trace:
        _cache["last_results"] = res
    y = np.zeros((B, N, D), np.float32)
    jac = np.zeros((B, N, D), np.float32)
    for b in range(B):
        o = res.results[b]["out"].reshape(2, D, N)
        y[b] = o[0].T + np.float32(b3)
        jac[b] = o[1].T
    return y, jac
